# revision 17
# baseline (speedup 1.0000x reference)
"""Self-contained Trainium2 Bass kernel for GQA attention (B=2, T=2048, D=4096,
32 q heads / 8 kv heads, HD=128, RoPE, no causal mask, start_pos=0).

Sharding: 8 cores = 2 (batch) x 4 (head groups). Each core computes 8 q heads /
2 kv heads for one batch and a partial o-projection; the host sums the 4
partials per batch.

Everything on device works in "transposed" space (feature dim on partitions):
    qT = wqT.T @ xT          (per 128-row o-block, accumulated over 32 d-tiles)
    scoresT[s,t] = kT(:,s).T @ qT      (K = head dim = 128, single tile)
    expT = exp(scoresT / sqrt(128))    (no max subtraction; |scores| <~ 12)
    den[t] = ones.T @ expT             (partition sums via M=1 matmul)
    ctxT = v_nat.T @ expT              (v projected directly in [t, hd] layout)
    ctxT *= broadcast(1/den)
    yT_partial = woT.T @ ctxT

RoPE: wq/wk rows are permuted on the host so each head's (re, im) pairs sit 16
partitions apart within a 32-partition quadrant; stream_shuffle swaps them and
two multiplies + add with host-built cos/sin tables apply the rotation.

Matmuls run in float32r (TF32-like single-pass PE mode).
"""

import sys
import math

for _p in ("/opt/trn_rl_repo", "/root/.axon_site"):
    if _p not in sys.path:
        sys.path.insert(0, _p)

import numpy as np

T = 2048
D = 4096
N_HEADS = 32
N_KV = 8
HD = 128
N_CORES = 8
GQ = N_HEADS // 4   # q heads per core = 8
GKV = N_KV // 4     # kv heads per core = 2
TCH = 512           # t-chunk for phase 1/2/3
SCALE = 1.0 / math.sqrt(HD)


def _build_program():
    import concourse.bass as bass
    import concourse.tile as tile
    from concourse import bacc, mybir

    f32 = mybir.dt.float32
    f32r = mybir.dt.float32r

    QD, KD, KT = GQ * HD, GKV * HD, D // 128
    N_REP = GQ // GKV

    nc = bacc.Bacc("TRN2", target_bir_lowering=False, debug=False,
                   num_devices=N_CORES)

    xT = nc.dram_tensor("xT", [D, T], f32r, kind="ExternalInput")
    wqT = nc.dram_tensor("wqT", [D, QD], f32r, kind="ExternalInput")
    wkvT = nc.dram_tensor("wkvT", [D, 2 * KD], f32r, kind="ExternalInput")
    woT = nc.dram_tensor("woT", [QD, D], f32r, kind="ExternalInput")
    C2 = nc.dram_tensor("C2", [128, T], f32, kind="ExternalInput")
    S2m = nc.dram_tensor("S2m", [128, T], f32, kind="ExternalInput")
    ones = nc.dram_tensor("ones", [128, 128], f32r, kind="ExternalInput")
    yT = nc.dram_tensor("yT", [D, T], f32, kind="ExternalOutput")

    NTCH = T // TCH
    NSB = T // 128            # s-blocks for attention
    SWAP = [(i + 16) % 32 for i in range(32)]  # swap 16-halves in each quadrant

    with tile.TileContext(nc) as tc:
        with tc.tile_pool(name="persist", bufs=1) as persist, \
             tc.tile_pool(name="dram", bufs=1, space="DRAM") as dram:
            ones_sb = persist.tile([128, 128], f32r, tag="ones")
            nc.sync.dma_start(ones_sb[:], ones[:])
            # kT (rope'd) per kv head, v in natural [t, hd] layout per t-block
            k_sb = [persist.tile([128, T], f32r, name=f"k{m}", tag=f"k{m}") for m in range(GKV)]
            v_sb = [persist.tile([128, KD], f32r, name=f"v{tb}", tag=f"v{tb}") for tb in range(T // 128)]
            q_dram = dram.tile([QD, T], f32r, tag="q_dram")

            # ---------------- Phase 1: q/k/v projections + RoPE ----------
            with tc.tile_pool(name="xt", bufs=1) as xtp, \
                 tc.tile_pool(name="wq", bufs=8) as wqp, \
                 tc.tile_pool(name="wkv", bufs=8) as wkvp, \
                 tc.tile_pool(name="rope", bufs=4) as ropep, \
                 tc.tile_pool(name="cst", bufs=1) as cstp, \
                 tc.tile_pool(name="p1ps", bufs=8, space="PSUM") as p1ps:
                c2_sb = cstp.tile([128, T], f32, tag="c2")
                nc.sync.dma_start(c2_sb[:], C2[:])
                s2m_sb = cstp.tile([128, T], f32, tag="s2m")
                nc.sync.dma_start(s2m_sb[:], S2m[:])

                def rope_evac(ps, dst_ap):
                    # dst = ps * C2 + shuffle(ps) * S2m  (on the tch column slice)
                    t1 = ropep.tile([128, TCH], f32, tag="t1")
                    nc.vector.tensor_mul(t1[:], ps[:], c2_sb[:, tcol0:tcol1])
                    sh = ropep.tile([128, TCH], f32, tag="sh")
                    nc.vector.stream_shuffle(sh[:], ps[:], SWAP)
                    t2 = ropep.tile([128, TCH], f32, tag="t2")
                    nc.vector.tensor_mul(t2[:], sh[:], s2m_sb[:, tcol0:tcol1])
                    nc.vector.tensor_add(dst_ap, t1[:], t2[:])

                for tch in range(NTCH):
                    tcol0, tcol1 = tch * TCH, (tch + 1) * TCH
                    xts = []
                    # pass A: q projection (8 live PSUM accumulators)
                    qps = [p1ps.tile([128, TCH], f32, name=f"qps{_}", tag="p1") for _ in range(GQ)]
                    for k in range(KT):
                        xt = xtp.tile([128, TCH], f32r, tag=f"xt{k}")
                        nc.sync.dma_start(xt[:], xT[k * 128:(k + 1) * 128, tcol0:tcol1])
                        xts.append(xt)
                        wq_sl = wqp.tile([128, QD], f32r, tag="wq")
                        nc.sync.dma_start(wq_sl[:], wqT[k * 128:(k + 1) * 128, :])
                        for m in range(GQ):
                            nc.tensor.matmul(qps[m][:], wq_sl[:, m * 128:(m + 1) * 128],
                                             xt[:], start=(k == 0), stop=(k == KT - 1))
                    for m in range(GQ):
                        qout = ropep.tile([128, TCH], f32r, tag="qout")
                        rope_evac(qps[m], qout[:])
                        nc.sync.dma_start(q_dram[m * 128:(m + 1) * 128, tcol0:tcol1], qout[:])
                    # pass B: k and v projections
                    kps = [p1ps.tile([128, TCH], f32, name=f"kps{_}", tag="p1") for _ in range(GKV)]
                    vps = [p1ps.tile([128, KD], f32, name=f"vps{_}", tag="p1") for _ in range(TCH // 128)]
                    for k in range(KT):
                        wkv_sl = wkvp.tile([128, 2 * KD], f32r, tag="wkv")
                        nc.sync.dma_start(wkv_sl[:], wkvT[k * 128:(k + 1) * 128, :])
                        for m in range(GKV):
                            nc.tensor.matmul(kps[m][:], wkv_sl[:, m * 128:(m + 1) * 128],
                                             xts[k][:], start=(k == 0), stop=(k == KT - 1))
                        for tb in range(TCH // 128):
                            nc.tensor.matmul(vps[tb][:], xts[k][:, tb * 128:(tb + 1) * 128],
                                             wkv_sl[:, KD:], start=(k == 0), stop=(k == KT - 1))
                    for m in range(GKV):
                        rope_evac(kps[m], k_sb[m][:, tcol0:tcol1])
                    for tb in range(TCH // 128):
                        nc.scalar.copy(v_sb[(tch * TCH) // 128 + tb][:], vps[tb][:])

            # ---------------- Phase 2: attention per q head --------------
            # t in windows of 1024; scores psum is [128,1024] (2 banks), exp is
            # one wide ACT op; denominators are accumulated on DVE/GpSimd and
            # reduced across partitions with a single M=1 matmul per chunk.
            # PSUM: sc 2x2 + ctx 2 + den 2 = 8 banks.
            TQ = 2 * TCH
            NSUB = TQ // TCH          # 2
            with tc.tile_pool(name="ctxsb", bufs=1) as ctxp:
              ctx_sb = [ctxp.tile([128, T], f32r, name=f"ctx{h}", tag=f"ctx{h}")
                        for h in range(GQ)]
              with tc.tile_pool(name="qt", bufs=8) as qtp, \
                 tc.tile_pool(name="exp", bufs=6) as expp, \
                 tc.tile_pool(name="nrm", bufs=8) as nrmp, \
                 tc.tile_pool(name="scps", bufs=4, space="PSUM") as scps, \
                 tc.tile_pool(name="ctxps", bufs=2, space="PSUM") as ctxps, \
                 tc.tile_pool(name="denps", bufs=2, space="PSUM") as denps:
                for h in range(GQ):
                    kv = h // N_REP
                    for tq in range(T // TQ):
                        qts, ctx_list, den_list = [], [], []
                        for j in range(NSUB):
                            c0 = tq * TQ + j * TCH
                            qt = qtp.tile([128, TCH], f32r, tag="qt")
                            nc.sync.dma_start(qt[:], q_dram[h * 128:(h + 1) * 128, c0:c0 + TCH])
                            qts.append(qt)
                            ctx_list.append(ctxps.tile([128, TCH], f32, name=f"ctxps{j}", tag="ctx"))
                            den_list.append(denps.tile([128, TCH], f32, name=f"denps{j}", tag="den"))
                        for sb in range(NSB):
                            exs = []
                            for j in range(NSUB):
                                sc_ps = scps.tile([128, TCH], f32, tag="sc")
                                nc.tensor.matmul(sc_ps[:],
                                                 k_sb[kv][:, sb * 128:(sb + 1) * 128],
                                                 qts[j][:], start=True, stop=True)
                                ex = expp.tile([128, TCH], f32r, tag="ex")
                                nc.scalar.activation(ex[:], sc_ps[:],
                                                     mybir.ActivationFunctionType.Exp,
                                                     scale=SCALE)
                                exs.append(ex)
                            for j in range(NSUB):
                                nc.tensor.matmul(den_list[j][:], ones_sb[:], exs[j][:],
                                                 start=(sb == 0), stop=(sb == NSB - 1))
                            for j in range(NSUB):
                                nc.tensor.matmul(ctx_list[j][:], v_sb[sb][:, kv * 128:(kv + 1) * 128],
                                                 exs[j][:],
                                                 start=(sb == 0), stop=(sb == NSB - 1))
                        for j in range(NSUB):
                            c0 = tq * TQ + j * TCH
                            rb = nrmp.tile([128, TCH], f32, tag="rb")
                            nc.vector.reciprocal(rb[:], den_list[j][:])
                            nc.vector.tensor_mul(ctx_sb[h][:, c0:c0 + TCH], ctx_list[j][:], rb[:])

              # ------------- Phase 3: o-projection (ctx in SBUF) -------
                with tc.tile_pool(name="wo", bufs=2) as wop, \
                     tc.tile_pool(name="out", bufs=8) as outp, \
                     tc.tile_pool(name="yps", bufs=8, space="PSUM") as yps:
                    for m in range(D // 128):
                        wo_t = []
                        for hk in range(GQ):
                            w = wop.tile([128, 128], f32r, name=f"wo{hk}", tag=f"wo{hk}")
                            nc.sync.dma_start(w[:], woT[hk * 128:(hk + 1) * 128,
                                                        m * 128:(m + 1) * 128])
                            wo_t.append(w)
                        y_list = [yps.tile([128, TCH], f32, name=f"yps{t_}", tag="y")
                                  for t_ in range(NTCH)]
                        for hk in range(GQ):
                            for tch in range(NTCH):
                                nc.tensor.matmul(y_list[tch][:], wo_t[hk][:],
                                                 ctx_sb[hk][:, tch * TCH:(tch + 1) * TCH],
                                                 start=(hk == 0), stop=(hk == GQ - 1))
                        for tch in range(NTCH):
                            ot = outp.tile([128, TCH], f32, tag="ot")
                            nc.scalar.copy(ot[:], y_list[tch][:])
                            nc.sync.dma_start(yT[m * 128:(m + 1) * 128,
                                                 tch * TCH:(tch + 1) * TCH], ot[:])

    nc.compile()
    return nc


_PROGRAM = None


def _get_program():
    global _PROGRAM
    if _PROGRAM is None:
        _PROGRAM = _build_program()
    return _PROGRAM


def _rope_perm():
    """Within-head row permutation: row 32*q + i  <-  component 2*(16q+i%16)+ (i>=16)."""
    perm = np.empty(HD, dtype=np.int64)
    for q in range(4):
        for i in range(32):
            j = 16 * q + (i % 16)
            perm[32 * q + i] = 2 * j + (1 if i >= 16 else 0)
    return perm


def _host_prep(x, wq, wk, wv, wo, cos, sin):
    """Build the per-core input maps."""
    perm = _rope_perm()
    f32 = np.float32
    QD, KD = GQ * HD, GKV * HD

    cosT = np.ascontiguousarray(cos.T.astype(f32))   # [64, T]
    sinT = np.ascontiguousarray(sin.T.astype(f32))
    C2 = np.empty((128, T), f32)
    S2m = np.empty((128, T), f32)
    for q in range(4):
        for i in range(32):
            j = 16 * q + (i % 16)
            C2[32 * q + i] = cosT[j]
            S2m[32 * q + i] = sinT[j] if i >= 16 else -sinT[j]
    ones = np.ones((128, 128), f32)

    in_maps = []
    for core in range(N_CORES):
        b, g = divmod(core, 4)
        qrows = np.concatenate([(8 * g + j) * HD + perm for j in range(GQ)])
        krows = np.concatenate([(2 * g + m) * HD + perm for m in range(GKV)])
        vrows = np.arange(2 * g * HD, (2 * g + 2) * HD)
        ocols = np.arange(8 * g * HD, (8 * g + 8) * HD)
        in_maps.append({
            "xT": np.ascontiguousarray(x[b].T.astype(f32)),
            "wqT": np.ascontiguousarray(wq[qrows].T.astype(f32)),
            "wkvT": np.ascontiguousarray(
                np.concatenate([wk[krows], wv[vrows]], axis=0).T.astype(f32)),
            "woT": np.ascontiguousarray(wo[:, ocols].T.astype(f32)),
            "C2": C2, "S2m": S2m, "ones": ones,
        })
    return in_maps


def kernel(x, wq, wk, wv, wo, cache_k, cache_v, cos, sin, mask, start_pos):
    x = np.asarray(x)
    wq, wk, wv, wo = (np.asarray(a) for a in (wq, wk, wv, wo))
    cos, sin = np.asarray(cos), np.asarray(sin)
    assert int(start_pos) == 0, "kernel hardcodes start_pos == 0"
    assert x.shape == (2, T, D)

    from concourse.bass_utils import run_bass_kernel_spmd

    nc = _get_program()
    in_maps = _host_prep(x, wq, wk, wv, wo, cos, sin)
    res = run_bass_kernel_spmd(nc, in_maps, list(range(N_CORES)))

    y = np.empty((2, T, D), np.float32)
    for b in range(2):
        acc = res.results[4 * b]["yT"].copy()
        for g in range(1, 4):
            acc += res.results[4 * b + g]["yT"]
        y[b] = acc.T
    return y
